# revision 1
# baseline (speedup 1.0000x reference)
"""Trainium2 Bass kernel for nn_AdditiveModel (grouped per-edge MLP + masked lag conv).

Reference computation (B=32768, N=16, L=16, H=16, G=N*N=256):
    xm  = x * (causal != 0)                     # [B, G, L]
    h1  = sigmoid(einsum('bgl,ghl->bgh', xm, W1) + b1)
    h2  = sigmoid(einsum('bgh,gkh->bgk', h1, W2) + b2)
    out = einsum('bvm,vm->bv', h2.reshape(B,N,N*H), W3) + b3   # [B, 16]

Strategy (pure data parallel over 8 NeuronCores, batch-sharded):
  - causal mask folded into W1 on the host; 8 groups packed block-diagonally
    per 128x128 weight tile so the per-group convs are full-width TensorE
    matmuls; W3 folded into a block-structured PSUM-accumulated stage 3.
  - The kernel is activation-bound: 2 sigmoids x 16.8M elements per core.
    ScalarE alone (1 elem/cycle/lane @1.2GHz + ~350cyc/instr overhead) costs
    ~294us.  To break that wall the sigmoid work is SPLIT between ScalarE and
    VectorE:
      * ScalarE: all stage-2 sigmoids + K_SCALAR of the 128 stage-1 tiles
        (exact, table-based ACT with fused bias).
      * VectorE: the remaining stage-1 tiles via two custom DVE ops:
          SIGEXP1P_ANT:  t = (cb - z/32)^32 + 1  ~= 1 + e^-(z+b1)
                         (FMA + 5 chained squarings + add = 8/8 ALU stages)
          RECIPROCAL_APPROX_FAST (stock): h1 = 1/t  (~51 ULP), bf16 out.
        The (1-u/32)^32 exponential underestimates by ~u^2/64 which perturbs
        h1 by <9e-3 absolute; filtered through W2 + sigmoid' this moves the
        final output rel-err from 1.88e-3 to only 1.91e-3 (verified vs
        reference) -- far inside the 2e-2 gate.  Custom ops are registered at
        import time (runtime uops-sha bootstrap), so kernel.py stays
        self-contained.
  - x and W1 are shipped bf16 (halves the dominant x DMA stream); stage-1
    matmul runs bf16 x bf16 at full PE rate.  h1/h2/W2/W3 bf16, PSUM fp32.
  - b1 rides on ScalarE's ACT bias; for DVE tiles it is pre-folded on the
    host into cb = 1 - b1/32 (per-partition scalar of SIGEXP1P_ANT).
  - x is host-bricked channel-major so every x DMA is fully contiguous.
  - the trace is software-pipelined (stage1 two iters ahead, stage3 one
    behind); the two act engines drain different PSUM banks in parallel.
"""

import sys
import time

import numpy as np

import ml_dtypes

if "/opt/trn_rl_repo" not in sys.path:
    sys.path.insert(0, "/opt/trn_rl_repo")

N = 16
L = 16
H = 16
B = 32768
G = N * N                 # 256 groups
NCORES = 8
BS = B // NCORES          # 4096 batch rows per core
C = G * L                 # 4096 channels (also G*H)
NCHUNK = 32               # channel chunks of 128
GRP = 8                   # groups per chunk
NBT = 4                   # batch tiles per core
BT = 1024                 # batch-tile width (columns)
NITER = NBT * NCHUNK      # 128 iterations per core

# stage-1 sigmoid engine split: K_SCALAR of the 128 stage-1 tiles stay on
# ScalarE (exact sigmoid); the rest go whole to VectorE.  Whole-tile
# ownership matters: splitting one PSUM tile between both act engines makes
# the Tile tracker serialize the reads (measured 280us vs 247us).
K_SCALAR = 53

_graph_cache = {}


def _register_dve_ops():
    """Register the custom SIGEXP1P_ANT op (idempotent).

    Computes t = (s0 - in0*s1)^32 + 1 where s0 is a per-partition scalar
    (cb = 1 - b1/32) and s1 = 1/32: an 8-stage fused approximation of
    1 + exp(-(z + b1)).  uops_sha is bootstrapped at runtime so this works
    against any repo checkout.
    """
    from concourse import dve_ops
    from concourse.dve_spec import Spec, Src0, C0, C1, One, sq, lower
    from concourse.dve_spec import _has_src1 as has_src1
    from concourse.dve_uop import DveOpSpec

    if "SIGEXP1P_ANT" in dve_ops._SUB_OPCODE_FOR_NAME:
        return next(o for o in dve_ops.OPS if o.name == "SIGEXP1P_ANT")

    def _ref(in0, in1, s0, s1, imm2):
        v = (s0 - in0 * s1).astype(np.float32)
        for _ in range(5):
            v = (v * v).astype(np.float32)
        return v + np.float32(1.0)

    body = sq(sq(sq(sq(sq(C0 - Src0 * C1))))) + One
    op = dve_ops.DveOp(
        "SIGEXP1P_ANT",
        Spec(body=body, reference=_ref),
        subdim=False,
        uops_sha={},
    )
    dve_ops.OPS.append(op)
    dve_ops._SUB_OPCODE_FOR_NAME[op.name] = (
        dve_ops._CUSTOM_DVE_ROW_BASE + len(dve_ops.OPS) - 1
    )
    dve_ops.CUSTOM_DVE_SPECS[op.name] = op.spec
    shas = {}
    for ver in ("v3", "v4"):
        spec_c = DveOpSpec(
            name=op.name,
            opcode=dve_ops.get_dve_sub_opcode(op.name),
            uops=lower(op.spec, ver=ver),
            rd1_en=has_src1(op.spec),
        )
        shas[ver] = spec_c.sha(ver)
    object.__setattr__(op, "uops_sha", shas)
    return op


def _scalar_s1_iters():
    """Evenly spread K_SCALAR ScalarE-owned stage-1 tiles over the run."""
    return {
        t for t in range(NITER)
        if ((t + 1) * K_SCALAR) // NITER > (t * K_SCALAR) // NITER
    }


def _build_graph():
    """Build + compile the per-core Bass graph (shared SPMD across 8 cores)."""
    from concourse import bacc, tile, mybir
    from concourse.dve_ops import RECIPROCAL_APPROX_FAST, RECIP_APPROX_FAST_CONSTS

    sig_op = _register_dve_ops()
    rc = RECIP_APPROX_FAST_CONSTS

    f32 = mybir.dt.float32
    bf16 = mybir.dt.bfloat16
    SIG = mybir.ActivationFunctionType.Sigmoid

    nc = bacc.Bacc("TRN2", target_bir_lowering=False, debug=False,
                   num_devices=NCORES)

    # x: [bt, cg, p, (j, col)] -- host-bricked so each [128, 4096] tile is a
    # single fully-contiguous 1 MiB DMA.  8 column-groups (cg) of 4 chunks (j).
    x_ext = nc.declare_dram_parameter("x", [NBT, 8, 128, 4096], bf16, isOutput=False)
    w1_ext = nc.declare_dram_parameter("w1", [128, NCHUNK * 128], bf16, isOutput=False)
    w2_ext = nc.declare_dram_parameter("w2", [128, NCHUNK * 128], bf16, isOutput=False)
    w3_ext = nc.declare_dram_parameter("w3", [128, NCHUNK * 16], bf16, isOutput=False)
    b1_ext = nc.declare_dram_parameter("b1", [128, NCHUNK], f32, isOutput=False)
    b1v_ext = nc.declare_dram_parameter("b1v", [128, NCHUNK], f32, isOutput=False)
    b2_ext = nc.declare_dram_parameter("b2", [128, NCHUNK], f32, isOutput=False)
    b3_ext = nc.declare_dram_parameter("b3", [16, 1], f32, isOutput=False)
    out_ext = nc.declare_dram_parameter("out", [16, BS], f32, isOutput=True)

    scalar_s1 = _scalar_s1_iters()

    with tile.TileContext(nc) as tc:
        with (
            tc.tile_pool(name="consts", bufs=1) as cpool,
            tc.tile_pool(name="xin", bufs=6) as xpool,
            tc.tile_pool(name="h1", bufs=4) as h1pool,
            tc.tile_pool(name="h2", bufs=4) as h2pool,
            tc.tile_pool(name="tsc", bufs=4) as tpool,
            tc.tile_pool(name="osb", bufs=2) as opool,
            tc.tile_pool(name="ps12", bufs=3, space="PSUM") as ps12pool,
            tc.tile_pool(name="ps3", bufs=1, space="PSUM") as ps3pool,
        ):
            xt = {}        # group idx -> x tile [128, 4096]

            def load_x_early(g):
                gbt, cg = divmod(g, 8)
                t = xpool.tile([128, 4096], bf16, tag="xin", name=f"x_{g}")
                nc.sync.dma_start(t[:], x_ext[gbt, cg])
                xt[g] = t

            # warm the sigmoid ACT table while the first DMAs stream
            warmsrc = cpool.tile([128, 1], f32)
            nc.vector.memset(warmsrc[:], 0.0)
            warm = cpool.tile([128, 1], f32)
            nc.scalar.activation(warm[:], warmsrc[:], SIG)
            # ramp: the very first matmul is gated only by w1 chunk 0
            # ([128,128]) and the first 512-col x strip; everything else
            # streams behind it.
            w1p = [cpool.tile([128, 8 * 128], bf16, name=f"w1p{i}")
                   for i in range(4)]
            w2p = [cpool.tile([128, 8 * 128], bf16, name=f"w2p{i}")
                   for i in range(4)]
            nc.sync.dma_start(w1p[0][:, 0:128], w1_ext[:, 0:128])
            x0s = []
            for s in range(2):
                xp = xpool.tile([128, 512], bf16, tag="xin", name=f"x0s{s}")
                nc.sync.dma_start(xp[:], x_ext[0, 0, :, s * 512:(s + 1) * 512])
                x0s.append(xp)
            nc.sync.dma_start(w1p[0][:, 128:1024], w1_ext[:, 128:1024])
            b1sb = cpool.tile([128, NCHUNK], f32)
            nc.sync.dma_start(b1sb[:], b1_ext[:])
            b1vsb = cpool.tile([128, NCHUNK], f32)
            nc.sync.dma_start(b1vsb[:], b1v_ext[:])
            nc.sync.dma_start(w2p[0][:], w2_ext[:, 0:1024])
            b2sb = cpool.tile([128, NCHUNK], f32)
            nc.sync.dma_start(b2sb[:], b2_ext[:])
            x0_parts = list(x0s)
            for j in range(1, 4):
                xp = xpool.tile([128, 1024], bf16, tag="xin", name=f"x0_{j}")
                nc.sync.dma_start(xp[:], x_ext[0, 0, :, j * 1024:(j + 1) * 1024])
                x0_parts.append(xp)
            xt[0] = x0_parts
            load_x_early(1)
            w3sb = cpool.tile([128, NCHUNK * 16], bf16)
            nc.sync.dma_start(w3sb[:], w3_ext[:])
            b3sb = cpool.tile([16, 1], f32)
            nc.sync.dma_start(b3sb[:], b3_ext[:])

            def w1_of(c):
                return w1p[c // 8][:, (c % 8) * 128:(c % 8 + 1) * 128]

            def w2_of(c):
                return w2p[c // 8][:, (c % 8) * 128:(c % 8 + 1) * 128]

            def load_late_weights(t):
                if t in (1, 2, 3):
                    nc.sync.dma_start(w1p[t][:],
                                      w1_ext[:, t * 1024:(t + 1) * 1024])
                elif t in (4, 5, 6):
                    i = t - 3
                    nc.sync.dma_start(w2p[i][:],
                                      w2_ext[:, i * 1024:(i + 1) * 1024])
            ps1 = {}
            h1d = {}
            h2d = {}
            ps3 = [None] * NBT
            pend_b = []    # deferred RECIPROCAL ops: (t, tsc tile)

            load_x = load_x_early

            def mm_pair(ps, lhsT, rhs_of, start=True, stop=True):
                # second matmul of the pair reuses the PE-resident stationary
                # weights (skips its LDWEIGHTS)
                for h in range(2):
                    mm = nc.tensor.matmul(
                        ps[:, h * 512:(h + 1) * 512],
                        lhsT=lhsT,
                        rhs=rhs_of(h),
                        start=start, stop=stop,
                    )
                    if h == 1:
                        mm.ins.ldweights = False

            def s1mm(t):
                bt, c = divmod(t, NCHUNK)
                g, j = divmod(t, 4)
                if g not in xt:
                    load_x(g)
                xg = xt[g]
                if isinstance(xg, list):
                    def rhs_of(h):
                        s = 2 * j + h          # 512-col strip index, 0..7
                        if s < 2:
                            return xg[s][:, 0:512]
                        return xg[2 + (s - 2) // 2][:, ((s - 2) % 2) * 512:
                                                    ((s - 2) % 2) * 512 + 512]
                else:
                    rhs_of = lambda h: xg[:, j * BT + h * 512:
                                          j * BT + (h + 1) * 512]
                ps = ps12pool.tile([128, BT], f32, tag="ps12")
                mm_pair(ps, w1_of(c), rhs_of)
                ps1[t] = ps

            def flush_opB():
                while pend_b:
                    tp, tsc, h1 = pend_b.pop(0)
                    nc.vector._custom_dve(
                        RECIPROCAL_APPROX_FAST, out=h1[:], in0=tsc[:],
                        s0=float(rc["s0"]), s1=float(rc["s1"]),
                        imm2=float(rc["imm2"]),
                    )

            def s1act(t):
                bt, c = divmod(t, NCHUNK)
                ps = ps1.pop(t)
                h1 = h1pool.tile([128, BT], bf16, tag="h1")
                if t in scalar_s1:
                    nc.scalar.activation(h1[:], ps[:], SIG,
                                         bias=b1sb[:, c:c + 1])
                    flush_opB()
                else:
                    # finish the previous tile's reciprocal first so its h1
                    # (the older dependency) completes earlier
                    flush_opB()
                    tsc = tpool.tile([128, BT], f32, tag="tsc")
                    nc.vector._custom_dve(
                        sig_op, out=tsc[:], in0=ps[:],
                        s0=b1vsb[:, c:c + 1], s1=float(1.0 / 32.0),
                    )
                    pend_b.append((t, tsc, h1))
                h1d[t] = h1

            def s2(t):
                bt, c = divmod(t, NCHUNK)
                if t + 1 >= NITER:
                    flush_opB()
                ps = ps12pool.tile([128, BT], f32, tag="ps12")
                h1 = h1d.pop(t)
                mm_pair(ps, w2_of(c),
                        lambda h: h1[:, h * 512:(h + 1) * 512])
                h2 = h2pool.tile([128, BT], bf16, tag="h2")
                nc.scalar.activation(h2[:], ps[:], SIG, bias=b2sb[:, c:c + 1])
                h2d[t] = h2

            def s3(t):
                bt, c = divmod(t, NCHUNK)
                if c == 0:
                    ps3[bt] = ps3pool.tile([16, BT], f32, tag="ps3", name=f"ps3_{bt}")
                h2 = h2d.pop(t)
                for h in range(2):
                    mm = nc.tensor.matmul(
                        ps3[bt][:, h * 512:(h + 1) * 512],
                        lhsT=w3sb[:, c * 16:(c + 1) * 16],
                        rhs=h2[:, h * 512:(h + 1) * 512],
                        start=(c == 0), stop=(c == NCHUNK - 1),
                    )
                    if h == 1:
                        mm.ins.ldweights = False
                if c == NCHUNK - 1:
                    osb = opool.tile([16, BT], f32, tag="osb")
                    nc.vector.tensor_scalar_add(osb[:], ps3[bt][:],
                                                b3sb[:, 0:1])
                    nc.sync.dma_start(out_ext[:, bt * BT:(bt + 1) * BT],
                                      osb[:])

            # Software pipeline: stage1 runs 2 iterations ahead of stage2 and
            # stage3 trails one behind.  ScalarE's stream alternates
            # s1act(t+1) (for its share of tiles), s2-act(t); VectorE drains
            # the other stage-1 tiles in parallel from different PSUM banks.
            s1mm(0)
            s1mm(1)
            s1act(0)
            for t in range(NITER):
                load_late_weights(t)
                if t + 2 < NITER:
                    s1mm(t + 2)
                if t + 1 < NITER:
                    s1act(t + 1)
                s2(t)
                if t >= 1:
                    s3(t - 1)
            s3(NITER - 1)

    nc.compile()
    return nc


def _get_graph():
    if "nc" not in _graph_cache:
        _graph_cache["nc"] = _build_graph()
    return _graph_cache["nc"]


def _prep_shared(causal, W1, b1, W2, b2, W3, b3):
    """Host-side weight packing (replicated across cores)."""
    bf = ml_dtypes.bfloat16
    mask = (np.asarray(causal).reshape(G, L) != 0).astype(np.float32)
    W1m = np.asarray(W1, dtype=np.float32) * mask[:, None, :]   # [G, H, L]

    def blockdiag(blk):
        # blk: [G, K_in=16, M_out=16] -> [128 (gl*16+k), NCHUNK*128 (c*128+m)]
        bd = np.zeros((NCHUNK, GRP, 16, GRP, 16), dtype=np.float32)
        b5 = blk.reshape(NCHUNK, GRP, 16, 16)
        for gl in range(GRP):
            bd[:, gl, :, gl, :] = b5[:, gl]
        return np.ascontiguousarray(
            bd.reshape(NCHUNK, 128, 128).transpose(1, 0, 2).reshape(128, -1))

    w1h = blockdiag(W1m.transpose(0, 2, 1)).astype(bf)          # k=lag, m=h
    w2h = blockdiag(
        np.asarray(W2, dtype=np.float32).transpose(0, 2, 1)).astype(bf)

    W3f = np.asarray(W3, dtype=np.float32)                      # [N, N*H]
    w3bd = np.zeros((NCHUNK, 128, 16), dtype=np.float32)
    for c in range(NCHUNK):
        w3bd[c, :, c // 2] = W3f[c // 2, (c % 2) * 128:(c % 2) * 128 + 128]
    w3h = np.ascontiguousarray(
        w3bd.transpose(1, 0, 2).reshape(128, NCHUNK * 16)).astype(bf)

    b1h = np.ascontiguousarray(
        np.asarray(b1, dtype=np.float32).reshape(NCHUNK, 128).T)
    b1vh = np.ascontiguousarray(1.0 - b1h / 32.0).astype(np.float32)
    b2h = np.ascontiguousarray(
        np.asarray(b2, dtype=np.float32).reshape(NCHUNK, 128).T)
    b3h = np.ascontiguousarray(
        np.asarray(b3, dtype=np.float32).reshape(16, 1))
    return w1h, w2h, w3h, b1h, b1vh, b2h, b3h


def _prep_x_shard(x_shard):
    """[BS, G, L] -> bricked channel-major [NBT, 8, 128, 4096] bf16."""
    xs = np.asarray(x_shard, dtype=np.float32).reshape(BS, C).T  # [C, BS]
    x5 = xs.reshape(8, 4, 128, NBT, BT)       # [cg, j, p, bt, col]
    return np.ascontiguousarray(x5.transpose(3, 0, 2, 1, 4)
                                .reshape(NBT, 8, 128, 4 * BT)
                                .astype(ml_dtypes.bfloat16))


def _run(inputs, trace=False, trace_cores=None):
    from concourse.bass_utils import run_bass_kernel_spmd

    nc = _get_graph()
    w1h, w2h, w3h, b1h, b1vh, b2h, b3h = _prep_shared(
        inputs["causal"], inputs["W1"], inputs["b1"], inputs["W2"],
        inputs["b2"], inputs["W3"], inputs["b3"])
    x = np.asarray(inputs["x"], dtype=np.float32)
    in_maps = []
    for i in range(NCORES):
        in_maps.append({
            "x": _prep_x_shard(x[i * BS:(i + 1) * BS]),
            "w1": w1h, "w2": w2h, "w3": w3h,
            "b1": b1h, "b1v": b1vh, "b2": b2h, "b3": b3h,
        })
    res = None
    last_err = None
    for attempt in range(3):
        try:
            res = run_bass_kernel_spmd(
                nc, in_maps, list(range(NCORES)),
                trace=trace, trace_cores=trace_cores)
            break
        except Exception as e:  # transient NRT device wedge heals on rerun
            last_err = e
            time.sleep(2.0)
    if res is None:
        raise last_err
    out = np.empty((B, N), dtype=np.float32)
    for i in range(NCORES):
        out[i * BS:(i + 1) * BS] = res.results[i]["out"].T
    return out, res


def kernel(**inputs) -> np.ndarray:
    out, _ = _run(inputs, trace=False)
    return out



# revision 8
# speedup vs baseline: 1.2126x; 1.2126x over previous
"""Trainium2 Bass kernel for nn_AdditiveModel (grouped per-edge MLP + masked lag conv).

Reference computation (B=32768, N=16, L=16, H=16, G=N*N=256):
    xm  = x * (causal != 0)                     # [B, G, L]
    h1  = sigmoid(einsum('bgl,ghl->bgh', xm, W1) + b1)
    h2  = sigmoid(einsum('bgh,gkh->bgk', h1, W2) + b2)
    out = einsum('bvm,vm->bv', h2.reshape(B,N,N*H), W3) + b3   # [B, 16]

Strategy (pure data parallel over 8 NeuronCores, batch-sharded):
  - causal mask folded into W1 on the host; 8 groups packed block-diagonally
    per 128x128 weight tile so the per-group convs are full-width TensorE
    matmuls; W3 folded into a block-structured PSUM-accumulated stage 3.
  - The kernel is activation-bound: 2 sigmoids x 16.8M elements per core.
    ScalarE alone (1 elem/cycle/lane @1.2GHz + ~350cyc/instr overhead) costs
    ~294us.  To break that wall the sigmoid work is SPLIT between ScalarE and
    VectorE:
      * ScalarE: all stage-2 sigmoids + K_SCALAR of the 128 stage-1 tiles
        (exact, table-based ACT with fused bias).
      * VectorE: the remaining stage-1 tiles via two custom DVE ops:
          SIGEXP1P_ANT:  t = (cb - z/32)^32 + 1  ~= 1 + e^-(z+b1)
                         (FMA + 5 chained squarings + add = 8/8 ALU stages)
          RECIPROCAL_APPROX_FAST (stock): h1 = 1/t  (~51 ULP), bf16 out.
        The (1-u/32)^32 exponential underestimates by ~u^2/64 which perturbs
        h1 by <9e-3 absolute; filtered through W2 + sigmoid' this moves the
        final output rel-err from 1.88e-3 to only 1.91e-3 (verified vs
        reference) -- far inside the 2e-2 gate.  Custom ops are registered at
        import time (runtime uops-sha bootstrap), so kernel.py stays
        self-contained.
  - x and W1 are shipped bf16 (halves the dominant x DMA stream); stage-1
    matmul runs bf16 x bf16 at full PE rate.  h1/h2/W2/W3 bf16, PSUM fp32.
  - b1 rides on ScalarE's ACT bias; for DVE tiles it is pre-folded on the
    host into cb = 1 - b1/32 (per-partition scalar of SIGEXP1P_ANT).
  - x is host-bricked channel-major so every x DMA is fully contiguous.
  - the trace is software-pipelined (stage1 two iters ahead, stage3 one
    behind); the two act engines drain different PSUM banks in parallel.
"""

import sys
import time

import numpy as np

import ml_dtypes

if "/opt/trn_rl_repo" not in sys.path:
    sys.path.insert(0, "/opt/trn_rl_repo")

N = 16
L = 16
H = 16
B = 32768
G = N * N                 # 256 groups
NCORES = 8
BS = B // NCORES          # 4096 batch rows per core
C = G * L                 # 4096 channels (also G*H)
NCHUNK = 32               # channel chunks of 128
GRP = 8                   # groups per chunk
NBT = 4                   # batch tiles per core
BT = 1024                 # batch-tile width (columns)
NITER = NBT * NCHUNK      # 128 iterations per core

# stage-1 sigmoid engine split: K_SCALAR of the 128 stage-1 tiles stay on
# ScalarE (exact sigmoid); the rest go whole to VectorE.  Whole-tile
# ownership matters: splitting one PSUM tile between both act engines makes
# the Tile tracker serialize the reads (measured 280us vs 247us).
K_SCALAR = 53

_graph_cache = {}


def _register_dve_ops():
    """Register the custom SIGEXP1P_ANT op (idempotent).

    Computes t = (s0 - in0*s1)^32 + 1 where s0 is a per-partition scalar
    (cb = 1 - b1/32) and s1 = 1/32: an 8-stage fused approximation of
    1 + exp(-(z + b1)).  uops_sha is bootstrapped at runtime so this works
    against any repo checkout.
    """
    from concourse import dve_ops
    from concourse.dve_spec import Spec, Src0, C0, C1, One, sq, lower
    from concourse.dve_spec import _has_src1 as has_src1
    from concourse.dve_uop import DveOpSpec

    if "SIGEXP1P_ANT" in dve_ops._SUB_OPCODE_FOR_NAME:
        return next(o for o in dve_ops.OPS if o.name == "SIGEXP1P_ANT")

    def _ref(in0, in1, s0, s1, imm2):
        v = (s0 - in0 * s1).astype(np.float32)
        for _ in range(5):
            v = (v * v).astype(np.float32)
        return v + np.float32(1.0)

    body = sq(sq(sq(sq(sq(C0 - Src0 * C1))))) + One
    op = dve_ops.DveOp(
        "SIGEXP1P_ANT",
        Spec(body=body, reference=_ref),
        subdim=False,
        uops_sha={},
    )
    dve_ops.OPS.append(op)
    dve_ops._SUB_OPCODE_FOR_NAME[op.name] = (
        dve_ops._CUSTOM_DVE_ROW_BASE + len(dve_ops.OPS) - 1
    )
    dve_ops.CUSTOM_DVE_SPECS[op.name] = op.spec
    shas = {}
    for ver in ("v3", "v4"):
        spec_c = DveOpSpec(
            name=op.name,
            opcode=dve_ops.get_dve_sub_opcode(op.name),
            uops=lower(op.spec, ver=ver),
            rd1_en=has_src1(op.spec),
        )
        shas[ver] = spec_c.sha(ver)
    object.__setattr__(op, "uops_sha", shas)
    return op


def _scalar_s1_iters():
    """Evenly spread K_SCALAR ScalarE-owned stage-1 tiles over the run."""
    return {
        t for t in range(NITER)
        if ((t + 1) * K_SCALAR) // NITER > (t * K_SCALAR) // NITER
    }


def _build_graph():
    """Build + compile the per-core Bass graph (shared SPMD across 8 cores)."""
    from concourse import bacc, tile, mybir
    from concourse.dve_ops import RECIPROCAL_APPROX_FAST, RECIP_APPROX_FAST_CONSTS

    sig_op = _register_dve_ops()
    rc = RECIP_APPROX_FAST_CONSTS

    f32 = mybir.dt.float32
    bf16 = mybir.dt.bfloat16
    fp8 = mybir.dt.float8e4
    SIG = mybir.ActivationFunctionType.Sigmoid

    nc = bacc.Bacc("TRN2", target_bir_lowering=False, debug=False,
                   num_devices=NCORES)

    # x: [bt, cg, p, (j, col)] -- host-bricked so each [128, 4096] tile is a
    # single fully-contiguous 512 KiB DMA.  8 column-groups (cg) of 4 chunks
    # (j).  fp8e4m3: halves the dominant HBM stream vs bf16; the x
    # quantization error (~3% rel) perturbs h1 by <4e-3 which is far inside
    # the 2e-2 gate (verified).
    x_ext = nc.declare_dram_parameter("x", [NBT, 8, 128, 4096], fp8, isOutput=False)
    w1_ext = nc.declare_dram_parameter("w1", [128, NCHUNK * 128], bf16, isOutput=False)
    w2_ext = nc.declare_dram_parameter("w2", [128, NCHUNK * 128], bf16, isOutput=False)
    w3_ext = nc.declare_dram_parameter("w3", [128, NCHUNK * 16], bf16, isOutput=False)
    b1_ext = nc.declare_dram_parameter("b1", [128, NCHUNK], f32, isOutput=False)
    b1v_ext = nc.declare_dram_parameter("b1v", [128, NCHUNK], f32, isOutput=False)
    b2_ext = nc.declare_dram_parameter("b2", [128, NCHUNK], f32, isOutput=False)
    b3_ext = nc.declare_dram_parameter("b3", [16, 1], f32, isOutput=False)
    out_ext = nc.declare_dram_parameter("out", [16, BS], f32, isOutput=True)

    scalar_s1 = _scalar_s1_iters()

    with tile.TileContext(nc) as tc:
        with (
            tc.tile_pool(name="consts", bufs=1) as cpool,
            tc.tile_pool(name="xin", bufs=8) as xpool,
            tc.tile_pool(name="h1", bufs=4) as h1pool,
            tc.tile_pool(name="h2", bufs=4) as h2pool,
            tc.tile_pool(name="tsc", bufs=4) as tpool,
            tc.tile_pool(name="osb", bufs=2) as opool,
            tc.tile_pool(name="ps12", bufs=3, space="PSUM") as ps12pool,
            tc.tile_pool(name="ps3", bufs=1, space="PSUM") as ps3pool,
        ):
            xt = {}        # group idx -> x tile [128, 4096]

            def load_x_early(g):
                gbt, cg = divmod(g, 8)
                t = xpool.tile([128, 4096], fp8, tag="xin", name=f"x_{g}")
                nc.sync.dma_start(t[:], x_ext[gbt, cg])
                xt[g] = t

            # warm the sigmoid ACT table while the first DMAs stream
            warmsrc = cpool.tile([128, 1], f32)
            nc.vector.memset(warmsrc[:], 0.0)
            warm = cpool.tile([128, 1], f32)
            nc.scalar.activation(warm[:], warmsrc[:], SIG)
            # HAM warm-up: the PE clock gate defaults to K=4/8 (1.2 GHz) and
            # only releases after ~3.4us of sustained PE activity.  The real
            # pipeline can't start until the first x strip lands (~11us), so
            # without this the first ~28us of matmuls run at half clock
            # (measured).  Keep the PE busy with small dummy matmuls on a
            # zeroed tile while the DMAs stream; by the time real data
            # arrives the array is at 2.4 GHz.
            wsrc = cpool.tile([128, 128], bf16, name="warm_mm_src")
            nc.vector.memset(wsrc[:], 0.0)
            wps = ps3pool.tile([16, 128], f32, tag="ps3", name="warm_mm_ps")
            for _ in range(26):
                nc.tensor.matmul(wps[:], lhsT=wsrc[:, 0:16], rhs=wsrc[:],
                                 start=True, stop=True)
            # ramp: the very first matmul is gated only by w1 chunk 0
            # ([128,128]) and the first 512-col x strip; everything else
            # streams behind it.
            # Weight/bias streams ride the GpSimd DMA queue: each
            # DMA_DIRECT2D trigger costs ~0.7us on its issuing queue, and
            # keeping the Sync queue exclusively for x tiles lets the x
            # prefetch issue without queuing behind weight transfers.
            w1p = [cpool.tile([128, 8 * 128], bf16, name=f"w1p{i}")
                   for i in range(4)]
            w2p = [cpool.tile([128, 8 * 128], bf16, name=f"w2p{i}")
                   for i in range(4)]
            nc.gpsimd.dma_start(w1p[0][:, 0:128], w1_ext[:, 0:128])
            x0s = []
            for s in range(2):
                xp = xpool.tile([128, 512], fp8, tag="xin", name=f"x0s{s}")
                nc.sync.dma_start(xp[:], x_ext[0, 0, :, s * 512:(s + 1) * 512])
                x0s.append(xp)
            nc.gpsimd.dma_start(w1p[0][:, 128:1024], w1_ext[:, 128:1024])
            b1sb = cpool.tile([128, NCHUNK], f32)
            nc.gpsimd.dma_start(b1sb[:], b1_ext[:])
            b1vsb = cpool.tile([128, NCHUNK], f32)
            nc.gpsimd.dma_start(b1vsb[:], b1v_ext[:])
            nc.gpsimd.dma_start(w2p[0][:], w2_ext[:, 0:1024])
            b2sb = cpool.tile([128, NCHUNK], f32)
            nc.gpsimd.dma_start(b2sb[:], b2_ext[:])
            x0_parts = list(x0s)
            for j in range(1, 4):
                xp = xpool.tile([128, 1024], fp8, tag="xin", name=f"x0_{j}")
                nc.sync.dma_start(xp[:], x_ext[0, 0, :, j * 1024:(j + 1) * 1024])
                x0_parts.append(xp)
            xt[0] = x0_parts
            load_x_early(1)
            w3sb = cpool.tile([128, NCHUNK * 16], bf16)
            nc.gpsimd.dma_start(w3sb[:], w3_ext[:])
            b3sb = cpool.tile([16, 1], f32)
            nc.gpsimd.dma_start(b3sb[:], b3_ext[:])

            def w1_of(c):
                return w1p[c // 8][:, (c % 8) * 128:(c % 8 + 1) * 128]

            def w2_of(c):
                return w2p[c // 8][:, (c % 8) * 128:(c % 8 + 1) * 128]

            def load_late_weights(t):
                if t in (1, 2, 3):
                    nc.gpsimd.dma_start(w1p[t][:],
                                        w1_ext[:, t * 1024:(t + 1) * 1024])
                elif t in (4, 5, 6):
                    i = t - 3
                    nc.gpsimd.dma_start(w2p[i][:],
                                        w2_ext[:, i * 1024:(i + 1) * 1024])
            ps1 = {}
            h1d = {}
            h2d = {}
            ps3 = [None] * NBT
            pend_b = []    # deferred RECIPROCAL ops: (t, tsc tile)

            load_x = load_x_early

            def mm_pair(ps, lhsT, rhs_of, start=True, stop=True):
                # second matmul of the pair reuses the PE-resident stationary
                # weights (skips its LDWEIGHTS)
                for h in range(2):
                    mm = nc.tensor.matmul(
                        ps[:, h * 512:(h + 1) * 512],
                        lhsT=lhsT,
                        rhs=rhs_of(h),
                        start=start, stop=stop,
                    )
                    if h == 1:
                        mm.ins.ldweights = False

            def s1mm(t):
                bt, c = divmod(t, NCHUNK)
                g, j = divmod(t, 4)
                if g not in xt:
                    load_x(g)
                xg = xt[g]
                if isinstance(xg, list):
                    def rhs_of(h):
                        s = 2 * j + h          # 512-col strip index, 0..7
                        if s < 2:
                            return xg[s][:, 0:512]
                        return xg[2 + (s - 2) // 2][:, ((s - 2) % 2) * 512:
                                                    ((s - 2) % 2) * 512 + 512]
                else:
                    rhs_of = lambda h: xg[:, j * BT + h * 512:
                                          j * BT + (h + 1) * 512]
                ps = ps12pool.tile([128, BT], f32, tag="ps12")
                mm_pair(ps, w1_of(c), rhs_of)
                ps1[t] = ps

            def flush_opB():
                while pend_b:
                    tp, tsc, h1 = pend_b.pop(0)
                    nc.vector._custom_dve(
                        RECIPROCAL_APPROX_FAST, out=h1[:], in0=tsc[:],
                        s0=float(rc["s0"]), s1=float(rc["s1"]),
                        imm2=float(rc["imm2"]),
                    )

            def s1act(t):
                bt, c = divmod(t, NCHUNK)
                ps = ps1.pop(t)
                h1 = h1pool.tile([128, BT], bf16, tag="h1")
                if t in scalar_s1:
                    nc.scalar.activation(h1[:], ps[:], SIG,
                                         bias=b1sb[:, c:c + 1])
                    flush_opB()
                else:
                    # finish the previous tile's reciprocal first so its h1
                    # (the older dependency) completes earlier
                    flush_opB()
                    tsc = tpool.tile([128, BT], f32, tag="tsc")
                    nc.vector._custom_dve(
                        sig_op, out=tsc[:], in0=ps[:],
                        s0=b1vsb[:, c:c + 1], s1=float(1.0 / 32.0),
                    )
                    pend_b.append((t, tsc, h1))
                h1d[t] = h1

            def s2(t):
                bt, c = divmod(t, NCHUNK)
                if t + 1 >= NITER:
                    flush_opB()
                ps = ps12pool.tile([128, BT], f32, tag="ps12")
                h1 = h1d.pop(t)
                mm_pair(ps, w2_of(c),
                        lambda h: h1[:, h * 512:(h + 1) * 512])
                h2 = h2pool.tile([128, BT], bf16, tag="h2")
                nc.scalar.activation(h2[:], ps[:], SIG, bias=b2sb[:, c:c + 1])
                h2d[t] = h2

            def s3(t):
                bt, c = divmod(t, NCHUNK)
                if c == 0:
                    ps3[bt] = ps3pool.tile([16, BT], f32, tag="ps3", name=f"ps3_{bt}")
                h2 = h2d.pop(t)
                for h in range(2):
                    mm = nc.tensor.matmul(
                        ps3[bt][:, h * 512:(h + 1) * 512],
                        lhsT=w3sb[:, c * 16:(c + 1) * 16],
                        rhs=h2[:, h * 512:(h + 1) * 512],
                        start=(c == 0), stop=(c == NCHUNK - 1),
                    )
                    if h == 1:
                        mm.ins.ldweights = False
                if c == NCHUNK - 1:
                    osb = opool.tile([16, BT], f32, tag="osb")
                    nc.vector.tensor_scalar_add(osb[:], ps3[bt][:],
                                                b3sb[:, 0:1])
                    nc.sync.dma_start(out_ext[:, bt * BT:(bt + 1) * BT],
                                      osb[:])

            # Software pipeline: stage1 runs 2 iterations ahead of stage2 and
            # stage3 trails one behind.  ScalarE's stream alternates
            # s1act(t+1) (for its share of tiles), s2-act(t); VectorE drains
            # the other stage-1 tiles in parallel from different PSUM banks.
            s1mm(0)
            s1mm(1)
            s1act(0)
            for t in range(NITER):
                load_late_weights(t)
                # keep the x prefetch ~1 tile (4 iterations) ahead of the
                # consuming s1mm so a ~3us tile DMA never stalls the PE
                g4 = (t + 5) // 4
                if t + 5 < NITER and g4 not in xt:
                    load_x(g4)
                if t + 2 < NITER:
                    s1mm(t + 2)
                if t + 1 < NITER:
                    s1act(t + 1)
                s2(t)
                if t >= 1:
                    s3(t - 1)
            s3(NITER - 1)

    nc.compile()
    return nc


def _get_graph():
    if "nc" not in _graph_cache:
        _graph_cache["nc"] = _build_graph()
    return _graph_cache["nc"]


def _prep_shared(causal, W1, b1, W2, b2, W3, b3):
    """Host-side weight packing (replicated across cores)."""
    bf = ml_dtypes.bfloat16
    mask = (np.asarray(causal).reshape(G, L) != 0).astype(np.float32)
    W1m = np.asarray(W1, dtype=np.float32) * mask[:, None, :]   # [G, H, L]

    def blockdiag(blk):
        # blk: [G, K_in=16, M_out=16] -> [128 (gl*16+k), NCHUNK*128 (c*128+m)]
        bd = np.zeros((NCHUNK, GRP, 16, GRP, 16), dtype=np.float32)
        b5 = blk.reshape(NCHUNK, GRP, 16, 16)
        for gl in range(GRP):
            bd[:, gl, :, gl, :] = b5[:, gl]
        return np.ascontiguousarray(
            bd.reshape(NCHUNK, 128, 128).transpose(1, 0, 2).reshape(128, -1))

    w1h = blockdiag(W1m.transpose(0, 2, 1)).astype(bf)          # k=lag, m=h
    w2h = blockdiag(
        np.asarray(W2, dtype=np.float32).transpose(0, 2, 1)).astype(bf)

    W3f = np.asarray(W3, dtype=np.float32)                      # [N, N*H]
    w3bd = np.zeros((NCHUNK, 128, 16), dtype=np.float32)
    for c in range(NCHUNK):
        w3bd[c, :, c // 2] = W3f[c // 2, (c % 2) * 128:(c % 2) * 128 + 128]
    w3h = np.ascontiguousarray(
        w3bd.transpose(1, 0, 2).reshape(128, NCHUNK * 16)).astype(bf)

    b1h = np.ascontiguousarray(
        np.asarray(b1, dtype=np.float32).reshape(NCHUNK, 128).T)
    b1vh = np.ascontiguousarray(1.0 - b1h / 32.0).astype(np.float32)
    b2h = np.ascontiguousarray(
        np.asarray(b2, dtype=np.float32).reshape(NCHUNK, 128).T)
    b3h = np.ascontiguousarray(
        np.asarray(b3, dtype=np.float32).reshape(16, 1))
    return w1h, w2h, w3h, b1h, b1vh, b2h, b3h


def _prep_x_shard(x_shard):
    """[BS, G, L] -> bricked channel-major [NBT, 8, 128, 4096] fp8e4m3."""
    xs = np.asarray(x_shard, dtype=np.float32).reshape(BS, C).T  # [C, BS]
    x5 = xs.reshape(8, 4, 128, NBT, BT)       # [cg, j, p, bt, col]
    return np.ascontiguousarray(x5.transpose(3, 0, 2, 1, 4)
                                .reshape(NBT, 8, 128, 4 * BT)
                                .astype(ml_dtypes.float8_e4m3fn))


def _run(inputs, trace=False, trace_cores=None):
    from concourse.bass_utils import run_bass_kernel_spmd

    nc = _get_graph()
    w1h, w2h, w3h, b1h, b1vh, b2h, b3h = _prep_shared(
        inputs["causal"], inputs["W1"], inputs["b1"], inputs["W2"],
        inputs["b2"], inputs["W3"], inputs["b3"])
    x = np.asarray(inputs["x"], dtype=np.float32)
    in_maps = []
    for i in range(NCORES):
        in_maps.append({
            "x": _prep_x_shard(x[i * BS:(i + 1) * BS]),
            "w1": w1h, "w2": w2h, "w3": w3h,
            "b1": b1h, "b1v": b1vh, "b2": b2h, "b3": b3h,
        })
    res = None
    last_err = None
    for attempt in range(3):
        try:
            res = run_bass_kernel_spmd(
                nc, in_maps, list(range(NCORES)),
                trace=trace, trace_cores=trace_cores)
            break
        except Exception as e:  # transient NRT device wedge heals on rerun
            last_err = e
            time.sleep(2.0)
    if res is None:
        raise last_err
    out = np.empty((B, N), dtype=np.float32)
    for i in range(NCORES):
        out[i * BS:(i + 1) * BS] = res.results[i]["out"].T
    return out, res


def kernel(**inputs) -> np.ndarray:
    out, _ = _run(inputs, trace=False)
    return out

